# revision 1
# baseline (speedup 1.0000x reference)
"""Trainium2 Bass kernel for nn_AttentionBlock (B=8, C=512, H=W=32, NH=8, DH=64).

Sharding: pure data-parallel — one batch element per NeuronCore (8 cores).
Per-core pipeline (channels-on-partitions layout, HW=1024 spatial):
  groupnorm -> qkv 1x1conv (bf16 matmul) -> attention:
    scores computed transposed (pT[j,i] = exp(k_j . q_i / 8), exp on ScalarE
    with the scale folded in, no softmax reductions needed), then
    out2T[d,i] = V^T-stationary matmul streaming pT (big 512-col streams),
    row sums via a ones-column in V, transpose via DMA xbar, normalize on
    VectorE -> reshape via DRAM round-trip -> proj 1x1conv -> residual.
The whole thing is software-pipelined over head pairs: pair h's scores/exp
run while pair h-1's attention-V matmuls run; the qkv/v convs fill pair 0's
slot and the proj fills the drain slot.

Host-side preprocessing: weights transposed + cast to bf16; v-bias and
proj-bias folded into a precomputed bias added to x for the residual.
"""

import numpy as np
import ml_dtypes

import concourse.bass as bass
import concourse.mybir as mybir
import concourse.tile as tile
from concourse import bacc
from concourse.bass_utils import run_bass_kernel_spmd

F32 = mybir.dt.float32
BF16 = mybir.dt.bfloat16
FP8E4 = mybir.dt.float8e4
FP8E5 = mybir.dt.float8e5

B, C, HW = 8, 512, 1024
NH, DH = 8, 64
GROUPS, EPS = 32, 1e-5
CT = C // 128          # 4 channel tiles
ST = HW // 128         # 8 spatial tiles
GPT = 8                # groups per 128-channel tile
CPG = 16               # channels per group

_CACHE: dict = {}


def _register_exp64():
    """Register the custom DVE op  out = (1 + in0*s0)^64  (approx exp(in0*s0*64)).

    1 mult + 1 add + 6 squarings = 8 ALU stages (the DVE datapath limit).
    The uop table is generated per-NEFF at compile time, no firmware change.
    """
    from concourse import dve_ops as DO
    if "EXP64_ANT" in DO._SUB_OPCODE_FOR_NAME:
        return next(op for op in DO.OPS if op.name == "EXP64_ANT")
    from concourse.dve_spec import Spec, Src0, C0, One, sq, lower, _has_src1
    from concourse.dve_uop import DveOpSpec

    body = sq(sq(sq(sq(sq(sq(One + Src0 * C0))))))
    spec = Spec(
        body=body,
        reference=lambda in0, in1, s0, s1, imm2:
            (1.0 + in0.astype(np.float32) * np.float32(s0)) ** 64,
    )
    row = max(DO._SUB_OPCODE_FOR_NAME.values()) + 1
    shas = {}
    for ver in ("v3", "v4"):
        try:
            u = lower(spec, ver=ver)
            shas[ver] = DveOpSpec(
                name="EXP64_ANT", opcode=row, uops=u, rd1_en=_has_src1(spec)
            ).sha(ver)
        except Exception:
            pass
    op = DO.DveOp("EXP64_ANT", spec, subdim=False, uops_sha=shas)
    DO.OPS.append(op)
    DO._SUB_OPCODE_FOR_NAME["EXP64_ANT"] = row
    DO.CUSTOM_DVE_SPECS["EXP64_ANT"] = spec
    return op




def _build():
    EXP64 = _register_exp64()
    nc = bacc.Bacc("TRN2", target_bir_lowering=False, debug=False, num_devices=8)

    x_d = nc.declare_dram_parameter("x", [C, HW], F32, isOutput=False)
    wq_d = nc.declare_dram_parameter("wqkvT", [C, 3 * C], BF16, isOutput=False)
    wp_d = nc.declare_dram_parameter("wprojT", [C, C], BF16, isOutput=False)
    qkb_d = nc.declare_dram_parameter("qkb", [2 * C], F32, isOutput=False)
    B2_d = nc.declare_dram_parameter("B2", [C, 64], F32, isOutput=False)
    gam_d = nc.declare_dram_parameter("gamma", [C], F32, isOutput=False)
    bet_d = nc.declare_dram_parameter("beta", [C], F32, isOutput=False)
    G_d = nc.declare_dram_parameter("G", [128, GPT], F32, isOutput=False)
    GT_d = nc.declare_dram_parameter("GT", [8, 128], F32, isOutput=False)
    out_d = nc.declare_dram_parameter("out", [C, HW], F32, isOutput=True)
    h2_d = nc.dram_tensor("h2d", [C, HW], BF16)

    import bass_rust
    from contextlib import ExitStack

    with tile.TileContext(nc) as tc, ExitStack() as ctx:
        const = ctx.enter_context(tc.tile_pool(name="const", bufs=1))
        small = ctx.enter_context(tc.tile_pool(name="small", bufs=2))
        xp = ctx.enter_context(tc.tile_pool(name="xp", bufs=1))
        hp = ctx.enter_context(tc.tile_pool(name="hp", bufs=1))
        wqp = ctx.enter_context(tc.tile_pool(name="wqp", bufs=1))
        wpp = ctx.enter_context(tc.tile_pool(name="wpp", bufs=1))
        qkp = ctx.enter_context(tc.tile_pool(name="qkp", bufs=1))
        vpl = ctx.enter_context(tc.tile_pool(name="vpl", bufs=1))
        ptp = ctx.enter_context(tc.tile_pool(name="ptp", bufs=4))
        o2tp = ctx.enter_context(tc.tile_pool(name="o2tp", bufs=2))
        o2trp = ctx.enter_context(tc.tile_pool(name="o2trp", bufs=2))
        o2p = ctx.enter_context(tc.tile_pool(name="o2p", bufs=2))
        h2p = ctx.enter_context(tc.tile_pool(name="h2p", bufs=1))
        outp = ctx.enter_context(tc.tile_pool(name="outp", bufs=2))
        # PSUM: psA = scores (+ proj at drain), psB = convs/attnv; 8 banks
        psA = ctx.enter_context(tc.tile_pool(name="psA", bufs=2, space="PSUM"))
        psB = ctx.enter_context(tc.tile_pool(name="psB", bufs=2, space="PSUM"))

        # ---- input DMAs, ordered so tile-0 compute can start early ----
        x_sb = xp.tile([128, CT, HW], F32)
        x_r = x_d[:].rearrange("(t p) s -> t p s", p=128)
        gam_sb = const.tile([128, CT], F32, tag="gam")
        nc.scalar.dma_start(out=gam_sb[:], in_=gam_d[:].rearrange("(t p) -> p t", p=128))
        bet_sb = const.tile([128, CT], F32, tag="bet")
        nc.scalar.dma_start(out=bet_sb[:], in_=bet_d[:].rearrange("(t p) -> p t", p=128))
        G_sb = const.tile([128, GPT], F32, tag="G")
        nc.scalar.dma_start(out=G_sb[:], in_=G_d[:])
        GT_sb = const.tile([8, 128], F32, tag="GT")
        nc.scalar.dma_start(out=GT_sb[:], in_=GT_d[:])
        nc.sync.dma_start(out=x_sb[:, 0, :], in_=x_r[0])
        nc.scalar.dma_start(out=x_sb[:, 1, :], in_=x_r[1])
        nc.sync.dma_start(out=x_sb[:, 2, :], in_=x_r[2])
        nc.scalar.dma_start(out=x_sb[:, 3, :], in_=x_r[3])
        wq_sb = wqp.tile([128, CT, 3 * C], BF16)
        wq_r = wq_d[:].rearrange("(t p) o -> t p o", p=128)
        for k in range(CT):
            nc.gpsimd.dma_start(out=wq_sb[:, k, :], in_=wq_r[k])
        qkb_sb = const.tile([128, 2 * CT], F32, tag="qkb")
        nc.scalar.dma_start(out=qkb_sb[:], in_=qkb_d[:].rearrange("(t p) -> p t", p=128))
        B2_sb = const.tile([128, CT, 64], F32, tag="B2")
        nc.scalar.dma_start(out=B2_sb[:], in_=B2_d[:].rearrange("(t p) m -> p t m", p=128))
        wp_sb = wpp.tile([128, CT, C], BF16)
        nc.gpsimd.dma_start(out=wp_sb[:], in_=wp_d[:].rearrange("(t p) o -> p t o", p=128))
        xpb_sb = xp.tile([128, CT, HW], F32, tag="xpb")

        # preload ACT sqrt table while DMAs run
        dummy = small.tile([1, 1], F32, tag="dummy")
        nc.vector.memset(dummy[:], 1.0)
        dummy2 = small.tile([1, 1], F32, tag="dummy2")
        nc.scalar.activation(dummy2[:], dummy[:],
                             mybir.ActivationFunctionType.Sqrt, bias=0.0, scale=1.0)

        # ---- per-tile groupnorm ----
        eps_sb = small.tile([8, 1], F32, tag="eps")
        nc.vector.memset(eps_sb[:], float(EPS))
        h_sb = hp.tile([128, CT, HW], BF16)
        mv = small.tile([128, CT, 3], F32, tag="mv")
        for t in range(CT):
            st = small.tile([128, 2, 6], F32, tag="bnst")
            x3 = x_sb[:, t, :].rearrange("p (a f) -> p a f", a=2)
            nc.vector.bn_stats(st[:, 0, :], x3[:, 0, :])
            nc.vector.bn_stats(st[:, 1, :], x3[:, 1, :])
            nc.vector.bn_aggr(mv[:, t, 0:2], st[:])
            nc.vector.tensor_mul(mv[:, t, 2:3], mv[:, t, 0:1], mv[:, t, 0:1])
            psg = psB.tile([8, 3], F32, tag="att", name=f"g_{t}")
            nc.tensor.matmul(psg[:], lhsT=G_sb[:], rhs=mv[:, t, :],
                             start=True, stop=True)
            gst = small.tile([8, 3], F32, tag="gst")
            nc.vector.tensor_copy(gst[:], psg[:])
            sqv = small.tile([8, 2], F32, tag="sqv")
            nc.vector.tensor_mul(sqv[:, 0:1], gst[:, 0:1], gst[:, 0:1])
            nc.vector.tensor_add(sqv[:, 1:2], gst[:, 1:2], gst[:, 2:3])
            nc.vector.tensor_sub(sqv[:, 1:2], sqv[:, 1:2], sqv[:, 0:1])
            srt = small.tile([8, 1], F32, tag="srt")
            nc.scalar.activation(srt[:], sqv[:, 1:2],
                                 mybir.ActivationFunctionType.Sqrt,
                                 bias=eps_sb[:], scale=1.0)
            rstd = small.tile([8, 1], F32, tag="rstd")
            nc.vector.reciprocal(rstd[:], srt[:])
            gv2 = small.tile([8, 2], F32, tag="gv2")
            nc.vector.tensor_copy(gv2[:, 0:1], rstd[:])
            nc.vector.tensor_copy(gv2[:, 1:2], gst[:, 0:1])
            bc_ps = psB.tile([128, 2], F32, tag="att", name=f"bc_{t}")
            nc.tensor.matmul(bc_ps[:], lhsT=GT_sb[:], rhs=gv2[:],
                             start=True, stop=True)
            sc = small.tile([128, CT, 2], F32, tag="sc")
            nc.vector.tensor_mul(sc[:, t, 0:1], bc_ps[:, 0:1], gam_sb[:, t:t + 1])
            nc.vector.tensor_mul(sc[:, t, 1:2], bc_ps[:, 1:2], sc[:, t, 0:1])
            nc.vector.tensor_sub(sc[:, t, 1:2], bet_sb[:, t:t + 1], sc[:, t, 1:2])
            eng = nc.vector if t % 2 == 0 else nc.gpsimd
            eng.tensor_scalar(
                out=h_sb[:, t, :], in0=x_sb[:, t, :],
                scalar1=sc[:, t, 0:1], scalar2=sc[:, t, 1:2],
                op0=mybir.AluOpType.mult, op1=mybir.AluOpType.add)
        # preload ACT exp table after the sqrts
        dummy3 = small.tile([1, 1], F32, tag="dummy3")
        nc.scalar.activation(dummy3[:], dummy[:],
                             mybir.ActivationFunctionType.Exp, scale=1.0)

        qk_sb = qkp.tile([128, 2 * CT, HW], BF16)
        v_sb = vpl.tile([128, ST, NH * 66], FP8E4)
        nc.vector.memset(
            v_sb[:].rearrange("p m (h e) -> p m h e", e=66)[:, :, :, 64], 1.0)
        h2_sb = h2p.tile([128, CT, HW], BF16)

        def emit_qk_conv(m):
            ps = psB.tile([128, HW], F32, tag="att", name=f"qkps{m}")
            for k in range(CT):
                for n in range(2):
                    nc.tensor.matmul(
                        ps[:, n * 512:(n + 1) * 512],
                        lhsT=wq_sb[:, k, m * 128:(m + 1) * 128],
                        rhs=h_sb[:, k, n * 512:(n + 1) * 512],
                        start=(k == 0), stop=(k == CT - 1))
            nc.vector.tensor_scalar_add(qk_sb[:, m, :], ps[:], qkb_sb[:, m:m + 1])

        def emit_v_conv(m):
            psv = psB.tile([128, 512], F32, tag="att", name=f"vps{m}")
            for k in range(CT):
                nc.tensor.matmul(
                    psv[:],
                    lhsT=h_sb[:, k, m * 128:(m + 1) * 128],
                    rhs=wq_sb[:, k, 2 * C:3 * C],
                    start=(k == 0), stop=(k == CT - 1))
            nc.vector.tensor_copy(
                v_sb[:, m, :].rearrange("p (h e) -> p h e", e=66)[:, :, 0:64],
                psv[:].rearrange("p (h d) -> p h d", d=64))

        # q/k tiles for pair 0 first, so its scores can start immediately
        emit_qk_conv(0)
        emit_qk_conv(4)
        # remaining conv work, interleaved into pair 0's attnv slot below
        conv_work = [lambda m=m: emit_qk_conv(m) for m in (1, 5, 2, 6, 3, 7)]
        conv_work += [lambda m=m: emit_v_conv(m) for m in range(ST)]

        def emit_scores_step(cur, step):
            pss, ops_ = [], []
            for (h, pt) in cur:
                base = 64 * (h % 2)
                ps = psA.tile([128, HW], F32, tag="sc", name=f"scps{h}_{step}")
                pss.append(ps)
            for n in range(2):
                for (h, pt), ps in zip(cur, pss):
                    base = 64 * (h % 2)
                    kT = qk_sb[base:base + 64, CT + h // 2,
                               step * 128:(step + 1) * 128]
                    qT = qk_sb[base:base + 64, h // 2, :]
                    nc.tensor.matmul(
                        ps[:, n * 512:(n + 1) * 512], lhsT=kT,
                        rhs=qT[:, n * 512:(n + 1) * 512],
                        start=True, stop=True)
            (hA, ptA), (hB, ptB) = cur
            nc.scalar.activation(
                ptA[:, step, :], pss[0][:],
                mybir.ActivationFunctionType.Exp,
                scale=float(DH ** -0.5))
            nc.vector._custom_dve(
                EXP64, out=ptB[:, step, :], in0=pss[1][:],
                s0=float(DH ** -0.5) / 64.0)

        def emit_attnv_chunk(prev, step, state):
            # head A during steps 0-3, head B during 4-7; DoubleRow packs a
            # j-tile pair per matmul (fp8 weights 2-per-cell, K=256 virtual)
            h, pt = prev[step // 4]
            sm = step % 4
            if sm == 0:
                state["po"] = psB.tile([128, HW], F32, tag="att", name=f"po{h}")
            po = state["po"]
            jj = 2 * sm
            v2_ = v_sb[:].rearrange(
                "p m (hh e) -> p m hh e", e=66)[:, jj:jj + 2, h, 0:65]
            for n in range(2):
                nc.tensor.matmul(
                    po[0:65, n * 512:(n + 1) * 512],
                    lhsT=v2_,
                    rhs=pt[:, jj:jj + 2, n * 512:(n + 1) * 512],
                    start=(sm == 0), stop=(sm == 3),
                    perf_mode=mybir.MatmulPerfMode.DoubleRow)
            if sm == 3:
                o2t = o2tp.tile([80, HW], BF16, tag="o2t")
                nc.vector.tensor_copy(o2t[0:65, :], po[0:65, :])
                o2tr = o2trp.tile([128, ST, 80], BF16, tag="o2tr")
                nc.sync.dma_start_transpose(o2tr[:], o2t[:])
                linv = small.tile([128, ST], F32, tag="linv")
                nc.vector.reciprocal(linv[:], o2tr[:, :, 64])
                o2 = o2p.tile([128, 512], BF16, tag="o2")
                lap = linv[:]
                lbc = bass.AP(tensor=lap.tensor, offset=lap.offset,
                              ap=[[lap.ap[0][0], 128], [1, ST], [0, 64]])
                nc.vector.tensor_mul(
                    o2[:].rearrange("p (q d) -> p q d", d=64),
                    o2tr[:, :, 0:64], lbc)
                wr = nc.sync.dma_start(
                    out=h2_d[:].rearrange("c s -> (c s)")
                    [h * 65536:(h + 1) * 65536]
                    .rearrange("(q p d) -> p q d", p=128, d=64),
                    in_=o2[:].rearrange("p (q d) -> p q d", d=64))
                state.setdefault("wr", []).append(wr)
                # read back this head's 64 h2 rows right away
                k, half = h // 2, h % 2
                rd = nc.sync.dma_start(
                    out=h2_sb[64 * half:64 * half + 64, k, :],
                    in_=h2_d[h * 64:(h + 1) * 64, :])
                bass_rust.add_dep_helper(rd.ins, wr.ins, reason="h2 RAW")

        proj_pp = {}

        def emit_proj(o, ks, finish, pool=None, tag="sc"):
            if o not in proj_pp:
                proj_pp[o] = (pool or psA).tile([128, HW], F32, tag=tag,
                                                name=f"pp{o}")
            pp = proj_pp[o]
            for k in ks:
                for n in range(2):
                    nc.tensor.matmul(
                        pp[:, n * 512:(n + 1) * 512],
                        lhsT=wp_sb[:, k, o * 128:(o + 1) * 128],
                        rhs=h2_sb[:, k, n * 512:(n + 1) * 512],
                        start=(k == 0), stop=(k == CT - 1))
            if finish:
                ot = outp.tile([128, HW], F32, tag="ot")
                nc.vector.tensor_add(ot[:], pp[:], xpb_sb[:, o, :])
                nc.sync.dma_start(out=out_d[o * 128:(o + 1) * 128, :], in_=ot[:])
                del proj_pp[o]

        # ---- attention pair loop (software pipelined) ----
        prev = None
        for hp_i in range(5):
            cur = None
            if hp_i < 4:
                hA, hB = 2 * hp_i, 2 * hp_i + 1
                ptA = ptp.tile([128, ST, HW], FP8E5, tag="pt", name=f"pt{hA}")
                ptB = ptp.tile([128, ST, HW], FP8E5, tag="pt", name=f"pt{hB}")
                cur = [(hA, ptA), (hB, ptB)]
            state = {}
            for step in range(8):
                if cur is not None:
                    emit_scores_step(cur, step)
                if prev is not None:
                    emit_attnv_chunk(prev, step, state)
                elif conv_work:
                    # pair 0: fill the attnv slot with remaining conv tiles
                    conv_work.pop(0)()
                    if conv_work and step % 2 == 1:
                        conv_work.pop(0)()
                if hp_i == 4 and step in (1, 3, 5):
                    # proj partials for o=0,1 over k=0..2 while pair 3 drains
                    emit_proj(0, [(step - 1) // 2], finish=False)
                    emit_proj(1, [(step - 1) // 2], finish=False)
                if hp_i == 4 and step in (4, 5, 6):
                    # o=2 partials in the attnv-A psum slot (freed at step 4)
                    emit_proj(2, [step - 4], finish=False, pool=psB, tag="att")
                if hp_i == 4 and step == 7:
                    # o=3 partials run during head 7's normalize/h2 chain
                    emit_proj(3, [0, 1, 2], finish=False, pool=psB, tag="att")
            while prev is None and conv_work:
                conv_work.pop(0)()
            if hp_i == 0:
                for t in range(CT):
                    bt = B2_sb[:, t, :]
                    b_bc = bass.AP(tensor=bt.tensor, offset=bt.offset,
                                   ap=[[bt.ap[0][0], 128], [0, 16], [1, 64]])
                    nc.vector.tensor_add(xpb_sb[:, t, :], x_sb[:, t, :], b_bc)
            prev = cur

        # ---- proj finish ----
        emit_proj(0, [3], finish=True)
        emit_proj(1, [3], finish=True)
        emit_proj(2, [3], finish=True)
        emit_proj(3, [3], finish=True)

    nc.compile()
    return nc


def _host_prep(x, norm_gamma, norm_beta, qkv_w, qkv_b, proj_w, proj_b):
    x = np.asarray(x, dtype=np.float32).reshape(B, C, HW)
    qkv_w = np.asarray(qkv_w, dtype=np.float32)
    qkv_b = np.asarray(qkv_b, dtype=np.float32)
    proj_w = np.asarray(proj_w, dtype=np.float32)
    proj_b = np.asarray(proj_b, dtype=np.float32)

    wqkvT = np.ascontiguousarray(qkv_w.T).astype(ml_dtypes.bfloat16)
    wprojT = np.ascontiguousarray(proj_w.T).astype(ml_dtypes.bfloat16)
    qkb = np.ascontiguousarray(qkv_b[:2 * C])
    vb = qkv_b[2 * C:].astype(np.float64)          # [512]
    # B2[o, m] = proj_b[o] + sum_h (sum_{c' in head h} proj_w[o, 64h+c']) * vb[64h+m]
    psum_h = proj_w.astype(np.float64).reshape(C, NH, DH).sum(axis=2)   # [o, h]
    vb_hm = vb.reshape(NH, DH)                                          # [h, m]
    B2 = (proj_b.astype(np.float64)[:, None] + psum_h @ vb_hm).astype(np.float32)

    G = np.zeros((128, GPT), np.float32)
    for p in range(128):
        G[p, p // CPG] = 1.0 / CPG
    GT = np.zeros((8, 128), np.float32)
    for p in range(128):
        GT[p // CPG, p] = 1.0

    gamma = np.ascontiguousarray(norm_gamma, dtype=np.float32)
    beta = np.ascontiguousarray(norm_beta, dtype=np.float32)

    in_maps = []
    for b in range(B):
        in_maps.append({
            "x": np.ascontiguousarray(x[b]),
            "B2": np.ascontiguousarray(B2),
            "wqkvT": wqkvT, "wprojT": wprojT,
            "qkb": qkb, "gamma": gamma, "beta": beta,
            "G": G, "GT": GT,
        })
    return in_maps


def _run(inputs: dict, trace: bool = False, tmpdir=None):
    if "nc" not in _CACHE:
        _CACHE["nc"] = _build()
    nc = _CACHE["nc"]
    in_maps = _host_prep(**inputs)
    res = run_bass_kernel_spmd(nc, in_maps, core_ids=list(range(8)), trace=trace,
                               tmpdir=tmpdir)
    out = np.stack([r["out"] for r in res.results]).reshape(B, C, 32, 32)
    return out.astype(np.float32), res


def kernel(**inputs):
    out, _ = _run(inputs, trace=False)
    return out



# revision 4
# speedup vs baseline: 1.0352x; 1.0352x over previous
"""Trainium2 Bass kernel for nn_AttentionBlock (B=8, C=512, H=W=32, NH=8, DH=64).

Sharding: pure data-parallel — one batch element per NeuronCore (8 cores).
Per-core pipeline (channels-on-partitions layout, HW=1024 spatial):
  groupnorm -> qkv 1x1conv (fp8 DoubleRow matmul) -> attention:
    scores computed transposed (pT[j,i] = exp(k_j . q_i / 8), exp split
    between ScalarE ACT and a custom DVE EXP64 op, no softmax reductions),
    then out2T[d,i] = V^T-stationary matmul streaming pT, row sums via a
    ones-column in V, transpose via DMA xbar, normalize on GpSimd
    -> reshape via DRAM round-trip -> proj 1x1conv (bf16) -> residual.
Software-pipelined over head pairs; conv tiles fill pair 0's attnv slot,
proj partials fill the drain slot.

v2 changes vs v1 (148.9us):
  - x DMA split per half-tile + groupnorm starts per-tile on arrival
  - qkv convs in fp8e4 DoubleRow (half the matmul count)
  - qk bias add + attn-out PSUM->SBUF cast moved to ScalarE (ACT identity/copy)
  - o2 normalize + xpb residual-bias adds moved to GpSimd
  - ACT table loads forced to (sqrt, exp) order once each
  - scores matmuls head-major for LDWEIGHTS adjacency
"""

import numpy as np
import ml_dtypes

import concourse.bass as bass
import concourse.mybir as mybir
import concourse.tile as tile
from concourse import bacc
from concourse.bass_utils import run_bass_kernel_spmd

F32 = mybir.dt.float32
BF16 = mybir.dt.bfloat16
FP8E4 = mybir.dt.float8e4
FP8E5 = mybir.dt.float8e5

B, C, HW = 8, 512, 1024
NH, DH = 8, 64
GROUPS, EPS = 32, 1e-5
CT = C // 128          # 4 channel tiles
ST = HW // 128         # 8 spatial tiles
GPT = 8                # groups per 128-channel tile
CPG = 16               # channels per group

FP8_CONV = True        # qkv conv in fp8e4 DoubleRow (proj stays bf16)

_CACHE: dict = {}


def _register_exp64():
    """Register the custom DVE op  out = (1 + in0*s0)^64  (approx exp(in0*s0*64)).

    1 mult + 1 add + 6 squarings = 8 ALU stages (the DVE datapath limit).
    The uop table is generated per-NEFF at compile time, no firmware change.
    """
    from concourse import dve_ops as DO
    if "EXP64_ANT" in DO._SUB_OPCODE_FOR_NAME:
        return next(op for op in DO.OPS if op.name == "EXP64_ANT")
    from concourse.dve_spec import Spec, Src0, C0, One, sq, lower, _has_src1
    from concourse.dve_uop import DveOpSpec

    body = sq(sq(sq(sq(sq(sq(One + Src0 * C0))))))
    spec = Spec(
        body=body,
        reference=lambda in0, in1, s0, s1, imm2:
            (1.0 + in0.astype(np.float32) * np.float32(s0)) ** 64,
    )
    row = max(DO._SUB_OPCODE_FOR_NAME.values()) + 1
    shas = {}
    for ver in ("v3", "v4"):
        try:
            u = lower(spec, ver=ver)
            shas[ver] = DveOpSpec(
                name="EXP64_ANT", opcode=row, uops=u, rd1_en=_has_src1(spec)
            ).sha(ver)
        except Exception:
            pass
    op = DO.DveOp("EXP64_ANT", spec, subdim=False, uops_sha=shas)
    DO.OPS.append(op)
    DO._SUB_OPCODE_FOR_NAME["EXP64_ANT"] = row
    DO.CUSTOM_DVE_SPECS["EXP64_ANT"] = spec
    return op


def _build():
    EXP64 = _register_exp64()
    nc = bacc.Bacc("TRN2", target_bir_lowering=False, debug=False, num_devices=8)

    WQDT = FP8E4 if FP8_CONV else BF16
    x_d = nc.declare_dram_parameter("x", [C, HW], F32, isOutput=False)
    wq_d = nc.declare_dram_parameter("wqkvT", [C, 3 * C], WQDT, isOutput=False)
    wp_d = nc.declare_dram_parameter("wprojT", [C, C], BF16, isOutput=False)
    qkb_d = nc.declare_dram_parameter("qkb", [2 * C], F32, isOutput=False)
    B2_d = nc.declare_dram_parameter("B2", [C, 64], F32, isOutput=False)
    gam_d = nc.declare_dram_parameter("gamma", [C], F32, isOutput=False)
    bet_d = nc.declare_dram_parameter("beta", [C], F32, isOutput=False)
    G_d = nc.declare_dram_parameter("G", [128, GPT], F32, isOutput=False)
    GT_d = nc.declare_dram_parameter("GT", [8, 128], F32, isOutput=False)
    out_d = nc.declare_dram_parameter("out", [C, HW], F32, isOutput=True)
    h2_d = nc.dram_tensor("h2d", [C, HW], BF16)

    import bass_rust
    from contextlib import ExitStack

    with tile.TileContext(nc) as tc, ExitStack() as ctx:
        const = ctx.enter_context(tc.tile_pool(name="const", bufs=1))
        small = ctx.enter_context(tc.tile_pool(name="small", bufs=2))
        xp = ctx.enter_context(tc.tile_pool(name="xp", bufs=1))
        hp = ctx.enter_context(tc.tile_pool(name="hp", bufs=1))
        wqp = ctx.enter_context(tc.tile_pool(name="wqp", bufs=1))
        wpp = ctx.enter_context(tc.tile_pool(name="wpp", bufs=1))
        qkp = ctx.enter_context(tc.tile_pool(name="qkp", bufs=1))
        vpl = ctx.enter_context(tc.tile_pool(name="vpl", bufs=1))
        ptp = ctx.enter_context(tc.tile_pool(name="ptp", bufs=4))
        o2tp = ctx.enter_context(tc.tile_pool(name="o2tp", bufs=2))
        o2trp = ctx.enter_context(tc.tile_pool(name="o2trp", bufs=2))
        o2p = ctx.enter_context(tc.tile_pool(name="o2p", bufs=2))
        h2p = ctx.enter_context(tc.tile_pool(name="h2p", bufs=1))
        outp = ctx.enter_context(tc.tile_pool(name="outp", bufs=2))
        # PSUM: psA = scores (+ proj at drain), psB = convs/attnv; 8 banks
        psA = ctx.enter_context(tc.tile_pool(name="psA", bufs=2, space="PSUM"))
        psB = ctx.enter_context(tc.tile_pool(name="psB", bufs=2, space="PSUM"))

        # ---- ACT table preloads first: sqrt now, exp forced after last sqrt
        dummy = small.tile([1, 1], F32, tag="dummy")
        nc.vector.memset(dummy[:], 1.0)
        dummy2 = small.tile([1, 1], F32, tag="dummy2")
        nc.scalar.activation(dummy2[:], dummy[:],
                             mybir.ActivationFunctionType.Sqrt, bias=0.0, scale=1.0)

        # ---- input DMAs: tiny consts, then x half-tiles, then weights ----
        gam_sb = const.tile([128, CT], F32, tag="gam")
        nc.scalar.dma_start(out=gam_sb[:], in_=gam_d[:].rearrange("(t p) -> p t", p=128))
        bet_sb = const.tile([128, CT], F32, tag="bet")
        nc.scalar.dma_start(out=bet_sb[:], in_=bet_d[:].rearrange("(t p) -> p t", p=128))
        G_sb = const.tile([128, GPT], F32, tag="G")
        nc.scalar.dma_start(out=G_sb[:], in_=G_d[:])
        GT_sb = const.tile([8, 128], F32, tag="GT")
        nc.scalar.dma_start(out=GT_sb[:], in_=GT_d[:])
        qkb_sb = const.tile([128, 2 * CT], F32, tag="qkb")
        nc.scalar.dma_start(out=qkb_sb[:], in_=qkb_d[:].rearrange("(t p) -> p t", p=128))
        B2_sb = const.tile([128, CT, 64], F32, tag="B2")
        nc.scalar.dma_start(out=B2_sb[:], in_=B2_d[:].rearrange("(t p) m -> p t m", p=128))

        x_sb = xp.tile([128, CT, HW], F32)
        x_r = x_d[:].rearrange("(t p) s -> t p s", p=128)
        x_arr = []            # per (tile, half) DMA handles
        for t in range(CT):
            d0 = nc.sync.dma_start(out=x_sb[:, t, 0:512], in_=x_r[t][:, 0:512])
            d1 = nc.scalar.dma_start(out=x_sb[:, t, 512:1024],
                                     in_=x_r[t][:, 512:1024])
            x_arr.append((d0, d1))

        wq_sb = wqp.tile([128, CT, 3 * C], WQDT)
        wq_r = wq_d[:].rearrange("(t p) o -> t p o", p=128)
        for k in range(CT):
            nc.gpsimd.dma_start(out=wq_sb[:, k, :], in_=wq_r[k])
        wp_sb = wpp.tile([128, CT, C], BF16)
        nc.gpsimd.dma_start(out=wp_sb[:], in_=wp_d[:].rearrange("(t p) o -> p t o", p=128))
        xpb_sb = xp.tile([128, CT, HW], F32, tag="xpb")

        # ---- per-tile groupnorm (starts as each x tile arrives) ----
        eps_sb = small.tile([8, 1], F32, tag="eps")
        nc.vector.memset(eps_sb[:], float(EPS))
        HDT = FP8E4 if FP8_CONV else BF16
        h_sb = hp.tile([128, CT, HW], HDT)
        mv = small.tile([128, CT, 3], F32, tag="mv")
        last_sqrt = None
        for t in range(CT):
            st = small.tile([128, 2, 6], F32, tag="bnst")
            x3 = x_sb[:, t, :].rearrange("p (a f) -> p a f", a=2)
            nc.vector.bn_stats(st[:, 0, :], x3[:, 0, :])
            nc.vector.bn_stats(st[:, 1, :], x3[:, 1, :])
            nc.vector.bn_aggr(mv[:, t, 0:2], st[:])
            nc.vector.tensor_mul(mv[:, t, 2:3], mv[:, t, 0:1], mv[:, t, 0:1])
            psg = psB.tile([8, 3], F32, tag="att", name=f"g_{t}")
            nc.tensor.matmul(psg[:], lhsT=G_sb[:], rhs=mv[:, t, :],
                             start=True, stop=True)
            gst = small.tile([8, 3], F32, tag="gst")
            nc.vector.tensor_copy(gst[:], psg[:])
            sqv = small.tile([8, 2], F32, tag="sqv")
            nc.vector.tensor_mul(sqv[:, 0:1], gst[:, 0:1], gst[:, 0:1])
            nc.vector.tensor_add(sqv[:, 1:2], gst[:, 1:2], gst[:, 2:3])
            nc.vector.tensor_sub(sqv[:, 1:2], sqv[:, 1:2], sqv[:, 0:1])
            srt = small.tile([8, 1], F32, tag="srt")
            last_sqrt = nc.scalar.activation(
                srt[:], sqv[:, 1:2], mybir.ActivationFunctionType.Sqrt,
                bias=eps_sb[:], scale=1.0)
            rstd = small.tile([8, 1], F32, tag="rstd")
            nc.vector.reciprocal(rstd[:], srt[:])
            gv2 = small.tile([8, 2], F32, tag="gv2")
            nc.vector.tensor_copy(gv2[:, 0:1], rstd[:])
            nc.vector.tensor_copy(gv2[:, 1:2], gst[:, 0:1])
            bc_ps = psB.tile([128, 2], F32, tag="att", name=f"bc_{t}")
            nc.tensor.matmul(bc_ps[:], lhsT=GT_sb[:], rhs=gv2[:],
                             start=True, stop=True)
            sc = small.tile([128, CT, 2], F32, tag="sc")
            nc.vector.tensor_mul(sc[:, t, 0:1], bc_ps[:, 0:1], gam_sb[:, t:t + 1])
            nc.vector.tensor_mul(sc[:, t, 1:2], bc_ps[:, 1:2], sc[:, t, 0:1])
            nc.vector.tensor_sub(sc[:, t, 1:2], bet_sb[:, t:t + 1], sc[:, t, 1:2])
            # apply per half, Vector + GpSimd in parallel
            for n, eng in ((0, nc.vector), (1, nc.gpsimd)):
                eng.tensor_scalar(
                    out=h_sb[:, t, n * 512:(n + 1) * 512],
                    in0=x_sb[:, t, n * 512:(n + 1) * 512],
                    scalar1=sc[:, t, 0:1], scalar2=sc[:, t, 1:2],
                    op0=mybir.AluOpType.mult, op1=mybir.AluOpType.add)
        # preload ACT exp table after the last sqrt (forced order so Tile
        # can't hoist it between the sqrts and thrash the table RAM)
        dummy3 = small.tile([1, 1], F32, tag="dummy3")
        expd = nc.scalar.activation(dummy3[:], dummy[:],
                                    mybir.ActivationFunctionType.Exp, scale=1.0)
        bass_rust.add_dep_helper(expd.ins, last_sqrt.ins, reason="ACT table order")

        qk_sb = qkp.tile([128, 2 * CT, HW], BF16)
        v_sb = vpl.tile([128, ST, NH * 66], FP8E4)
        nc.vector.memset(
            v_sb[:].rearrange("p m (h e) -> p m h e", e=66)[:, :, :, 64], 1.0)
        h2_sb = h2p.tile([128, CT, HW], BF16)

        if FP8_CONV:
            def emit_qk_conv(m):
                ps = psB.tile([128, HW], F32, tag="att", name=f"qkps{m}")
                for kk in (0, 2):
                    for n in range(2):
                        nc.tensor.matmul(
                            ps[:, n * 512:(n + 1) * 512],
                            lhsT=wq_sb[:, kk:kk + 2, m * 128:(m + 1) * 128],
                            rhs=h_sb[:, kk:kk + 2, n * 512:(n + 1) * 512],
                            start=(kk == 0), stop=(kk == 2),
                            perf_mode=mybir.MatmulPerfMode.DoubleRow)
                nc.scalar.add(qk_sb[:, m, :], ps[:], add=qkb_sb[:, m:m + 1])

            def emit_v_conv(m):
                psv = psB.tile([128, 512], F32, tag="att", name=f"vps{m}")
                for kk in (0, 2):
                    nc.tensor.matmul(
                        psv[:],
                        lhsT=h_sb[:, kk:kk + 2, m * 128:(m + 1) * 128],
                        rhs=wq_sb[:, kk:kk + 2, 2 * C:3 * C],
                        start=(kk == 0), stop=(kk == 2),
                        perf_mode=mybir.MatmulPerfMode.DoubleRow)
                nc.vector.tensor_copy(
                    v_sb[:, m, :].rearrange("p (h e) -> p h e", e=66)[:, :, 0:64],
                    psv[:].rearrange("p (h d) -> p h d", d=64))
        else:
            def emit_qk_conv(m):
                ps = psB.tile([128, HW], F32, tag="att", name=f"qkps{m}")
                for k in range(CT):
                    for n in range(2):
                        nc.tensor.matmul(
                            ps[:, n * 512:(n + 1) * 512],
                            lhsT=wq_sb[:, k, m * 128:(m + 1) * 128],
                            rhs=h_sb[:, k, n * 512:(n + 1) * 512],
                            start=(k == 0), stop=(k == CT - 1))
                nc.scalar.add(qk_sb[:, m, :], ps[:], add=qkb_sb[:, m:m + 1])

            def emit_v_conv(m):
                psv = psB.tile([128, 512], F32, tag="att", name=f"vps{m}")
                for k in range(CT):
                    nc.tensor.matmul(
                        psv[:],
                        lhsT=h_sb[:, k, m * 128:(m + 1) * 128],
                        rhs=wq_sb[:, k, 2 * C:3 * C],
                        start=(k == 0), stop=(k == CT - 1))
                nc.vector.tensor_copy(
                    v_sb[:, m, :].rearrange("p (h e) -> p h e", e=66)[:, :, 0:64],
                    psv[:].rearrange("p (h d) -> p h d", d=64))

        # q/k tiles for pair 0 first, so its scores can start immediately
        emit_qk_conv(0)
        emit_qk_conv(4)
        # remaining conv work, interleaved into pair 0's attnv slot below
        conv_work = [lambda m=m: emit_qk_conv(m) for m in (1, 5, 2, 6, 3, 7)]
        conv_work += [lambda m=m: emit_v_conv(m) for m in range(ST)]

        def emit_scores_step(cur, step):
            pss = []
            for (h, pt) in cur:
                base = 64 * (h % 2)
                ps = psA.tile([128, HW], F32, tag="sc", name=f"scps{h}_{step}")
                pss.append(ps)
                kT = qk_sb[base:base + 64, CT + h // 2,
                           step * 128:(step + 1) * 128]
                qT = qk_sb[base:base + 64, h // 2, :]
                for n in range(2):
                    nc.tensor.matmul(
                        ps[:, n * 512:(n + 1) * 512], lhsT=kT,
                        rhs=qT[:, n * 512:(n + 1) * 512],
                        start=True, stop=True)
            (hA, ptA), (hB, ptB) = cur
            nc.scalar.activation(
                ptA[:, step, :], pss[0][:],
                mybir.ActivationFunctionType.Exp,
                scale=float(DH ** -0.5))
            nc.vector._custom_dve(
                EXP64, out=ptB[:, step, :], in0=pss[1][:],
                s0=float(DH ** -0.5) / 64.0)

        def emit_attnv_chunk(prev, step, state):
            # head A during steps 0-3, head B during 4-7; DoubleRow packs a
            # j-tile pair per matmul (fp8 weights 2-per-cell, K=256 virtual)
            h, pt = prev[step // 4]
            sm = step % 4
            if sm == 0:
                state["po"] = psB.tile([128, HW], F32, tag="att", name=f"po{h}")
            po = state["po"]
            jj = 2 * sm
            v2_ = v_sb[:].rearrange(
                "p m (hh e) -> p m hh e", e=66)[:, jj:jj + 2, h, 0:65]
            for n in range(2):
                nc.tensor.matmul(
                    po[0:65, n * 512:(n + 1) * 512],
                    lhsT=v2_,
                    rhs=pt[:, jj:jj + 2, n * 512:(n + 1) * 512],
                    start=(sm == 0), stop=(sm == 3),
                    perf_mode=mybir.MatmulPerfMode.DoubleRow)
            if sm == 3:
                o2t = o2tp.tile([80, HW], BF16, tag="o2t")
                nc.scalar.copy(o2t[0:65, :], po[0:65, :])
                o2tr = o2trp.tile([128, ST, 80], BF16, tag="o2tr")
                nc.sync.dma_start_transpose(o2tr[:], o2t[:])
                linv = small.tile([128, ST], F32, tag="linv")
                nc.vector.reciprocal(linv[:], o2tr[:, :, 64])
                o2 = o2p.tile([128, 512], BF16, tag="o2")
                lap = linv[:]
                lbc = bass.AP(tensor=lap.tensor, offset=lap.offset,
                              ap=[[lap.ap[0][0], 128], [1, ST], [0, 64]])
                nc.gpsimd.tensor_mul(
                    o2[:].rearrange("p (q d) -> p q d", d=64),
                    o2tr[:, :, 0:64], lbc)
                wr = nc.sync.dma_start(
                    out=h2_d[:].rearrange("c s -> (c s)")
                    [h * 65536:(h + 1) * 65536]
                    .rearrange("(q p d) -> p q d", p=128, d=64),
                    in_=o2[:].rearrange("p (q d) -> p q d", d=64))
                state.setdefault("wr", []).append(wr)
                # read back this head's 64 h2 rows right away
                k, half = h // 2, h % 2
                rd = nc.sync.dma_start(
                    out=h2_sb[64 * half:64 * half + 64, k, :],
                    in_=h2_d[h * 64:(h + 1) * 64, :])
                bass_rust.add_dep_helper(rd.ins, wr.ins, reason="h2 RAW")

        proj_pp = {}

        def emit_proj(o, ks, finish, pool=None, tag="sc"):
            if o not in proj_pp:
                proj_pp[o] = (pool or psA).tile([128, HW], F32, tag=tag,
                                                name=f"pp{o}")
            pp = proj_pp[o]
            for k in ks:
                for n in range(2):
                    nc.tensor.matmul(
                        pp[:, n * 512:(n + 1) * 512],
                        lhsT=wp_sb[:, k, o * 128:(o + 1) * 128],
                        rhs=h2_sb[:, k, n * 512:(n + 1) * 512],
                        start=(k == 0), stop=(k == CT - 1))
            if finish:
                ot = outp.tile([128, HW], F32, tag="ot")
                nc.vector.tensor_add(ot[:], pp[:], xpb_sb[:, o, :])
                eng = nc.sync if o % 2 == 0 else nc.scalar
                eng.dma_start(out=out_d[o * 128:(o + 1) * 128, :], in_=ot[:])
                del proj_pp[o]

        # ---- attention pair loop (software pipelined) ----
        prev = None
        for hp_i in range(5):
            cur = None
            if hp_i < 4:
                hA, hB = 2 * hp_i, 2 * hp_i + 1
                ptA = ptp.tile([128, ST, HW], FP8E5, tag="pt", name=f"pt{hA}")
                ptB = ptp.tile([128, ST, HW], FP8E5, tag="pt", name=f"pt{hB}")
                cur = [(hA, ptA), (hB, ptB)]
            state = {}
            for step in range(8):
                if cur is not None:
                    emit_scores_step(cur, step)
                if prev is not None:
                    emit_attnv_chunk(prev, step, state)
                elif conv_work:
                    # pair 0: fill the attnv slot with remaining conv tiles
                    conv_work.pop(0)()
                    if conv_work and step % 2 == 1:
                        conv_work.pop(0)()
                if hp_i == 4 and step in (1, 3, 5):
                    # proj partials for o=0,1 over k=0..2 while pair 3 drains
                    emit_proj(0, [(step - 1) // 2], finish=False)
                    emit_proj(1, [(step - 1) // 2], finish=False)
                if hp_i == 4 and step in (4, 5, 6):
                    # o=2 partials in the attnv-A psum slot (freed at step 4)
                    emit_proj(2, [step - 4], finish=False, pool=psB, tag="att")
                if hp_i == 4 and step == 7:
                    # o=3 partials run during head 7's normalize/h2 chain
                    emit_proj(3, [0, 1, 2], finish=False, pool=psB, tag="att")
            while prev is None and conv_work:
                conv_work.pop(0)()
            if hp_i == 0:
                for t in range(CT):
                    bt = B2_sb[:, t, :]
                    b_bc = bass.AP(tensor=bt.tensor, offset=bt.offset,
                                   ap=[[bt.ap[0][0], 128], [0, 16], [1, 64]])
                    nc.gpsimd.tensor_add(xpb_sb[:, t, :], x_sb[:, t, :], b_bc)
            prev = cur

        # ---- proj finish ----
        emit_proj(0, [3], finish=True)
        emit_proj(1, [3], finish=True)
        emit_proj(2, [3], finish=True)
        emit_proj(3, [3], finish=True)

    nc.compile()
    return nc


def _host_prep(x, norm_gamma, norm_beta, qkv_w, qkv_b, proj_w, proj_b):
    x = np.asarray(x, dtype=np.float32).reshape(B, C, HW)
    qkv_w = np.asarray(qkv_w, dtype=np.float32)
    qkv_b = np.asarray(qkv_b, dtype=np.float32)
    proj_w = np.asarray(proj_w, dtype=np.float32)
    proj_b = np.asarray(proj_b, dtype=np.float32)

    wq_np = np.ascontiguousarray(qkv_w.T)
    if FP8_CONV:
        wqkvT = np.clip(wq_np, -440.0, 440.0).astype(ml_dtypes.float8_e4m3fn)
    else:
        wqkvT = wq_np.astype(ml_dtypes.bfloat16)
    wprojT = np.ascontiguousarray(proj_w.T).astype(ml_dtypes.bfloat16)
    qkb = np.ascontiguousarray(qkv_b[:2 * C])
    vb = qkv_b[2 * C:].astype(np.float64)          # [512]
    # B2[o, m] = proj_b[o] + sum_h (sum_{c' in head h} proj_w[o, 64h+c']) * vb[64h+m]
    psum_h = proj_w.astype(np.float64).reshape(C, NH, DH).sum(axis=2)   # [o, h]
    vb_hm = vb.reshape(NH, DH)                                          # [h, m]
    B2 = (proj_b.astype(np.float64)[:, None] + psum_h @ vb_hm).astype(np.float32)

    G = np.zeros((128, GPT), np.float32)
    for p in range(128):
        G[p, p // CPG] = 1.0 / CPG
    GT = np.zeros((8, 128), np.float32)
    for p in range(128):
        GT[p // CPG, p] = 1.0

    gamma = np.ascontiguousarray(norm_gamma, dtype=np.float32)
    beta = np.ascontiguousarray(norm_beta, dtype=np.float32)

    in_maps = []
    for b in range(B):
        in_maps.append({
            "x": np.ascontiguousarray(x[b]),
            "B2": np.ascontiguousarray(B2),
            "wqkvT": wqkvT, "wprojT": wprojT,
            "qkb": qkb, "gamma": gamma, "beta": beta,
            "G": G, "GT": GT,
        })
    return in_maps


def _run(inputs: dict, trace: bool = False, tmpdir=None):
    if "nc" not in _CACHE:
        _CACHE["nc"] = _build()
    nc = _CACHE["nc"]
    in_maps = _host_prep(**inputs)
    res = run_bass_kernel_spmd(nc, in_maps, core_ids=list(range(8)), trace=trace,
                               tmpdir=tmpdir)
    out = np.stack([r["out"] for r in res.results]).reshape(B, C, 32, 32)
    return out.astype(np.float32), res


def kernel(**inputs):
    out, _ = _run(inputs, trace=False)
    return out


# revision 12
# speedup vs baseline: 1.1241x; 1.0859x over previous
"""Trainium2 Bass kernel for nn_AttentionBlock (B=8, C=512, H=W=32, NH=8, DH=64).

Sharding: pure data-parallel — one batch element per NeuronCore (8 cores).
Per-core pipeline (channels-on-partitions layout, HW=1024 spatial):
  groupnorm -> qkv 1x1conv (fp8 DoubleRow matmul) -> attention:
    scores computed transposed (pT[j,i] = exp(k_j . q_i / 8), exp split
    between ScalarE ACT and a custom DVE EXP64 op, no softmax reductions),
    then out2T[d,i] = V^T-stationary matmul streaming pT, row sums via a
    ones-column in V, transpose via DMA xbar, normalize on GpSimd
    -> reshape via DRAM round-trip -> proj 1x1conv (bf16) -> residual.
Software-pipelined over head pairs; conv tiles fill pair 0's attnv slot,
proj partials fill the drain slot.

v2 changes vs v1 (148.9us):
  - x DMA split per half-tile + groupnorm starts per-tile on arrival
  - qkv convs in fp8e4 DoubleRow (half the matmul count)
  - qk bias add + attn-out PSUM->SBUF cast moved to ScalarE (ACT identity/copy)
  - o2 normalize + xpb residual-bias adds moved to GpSimd
  - ACT table loads forced to (sqrt, exp) order once each
  - scores matmuls head-major for LDWEIGHTS adjacency
"""

import numpy as np
import ml_dtypes

import concourse.bass as bass
import concourse.mybir as mybir
import concourse.tile as tile
from concourse import bacc
from concourse.bass_utils import run_bass_kernel_spmd

F32 = mybir.dt.float32
BF16 = mybir.dt.bfloat16
FP8E4 = mybir.dt.float8e4
FP8E5 = mybir.dt.float8e5

B, C, HW = 8, 512, 1024
NH, DH = 8, 64
GROUPS, EPS = 32, 1e-5
CT = C // 128          # 4 channel tiles
ST = HW // 128         # 8 spatial tiles
GPT = 8                # groups per 128-channel tile
CPG = 16               # channels per group

FP8_CONV = True        # qkv conv in fp8e4 DoubleRow (proj stays bf16)

_CACHE: dict = {}


def _register_exp64():
    """Register the custom DVE op  out = (1 + in0*s0)^64  (approx exp(in0*s0*64)).

    1 mult + 1 add + 6 squarings = 8 ALU stages (the DVE datapath limit).
    The uop table is generated per-NEFF at compile time, no firmware change.
    """
    from concourse import dve_ops as DO
    if "EXP64_ANT" in DO._SUB_OPCODE_FOR_NAME:
        return next(op for op in DO.OPS if op.name == "EXP64_ANT")
    from concourse.dve_spec import Spec, Src0, C0, One, sq, lower, _has_src1
    from concourse.dve_uop import DveOpSpec

    body = sq(sq(sq(sq(sq(sq(One + Src0 * C0))))))
    spec = Spec(
        body=body,
        reference=lambda in0, in1, s0, s1, imm2:
            (1.0 + in0.astype(np.float32) * np.float32(s0)) ** 64,
    )
    row = max(DO._SUB_OPCODE_FOR_NAME.values()) + 1
    shas = {}
    for ver in ("v3", "v4"):
        try:
            u = lower(spec, ver=ver)
            shas[ver] = DveOpSpec(
                name="EXP64_ANT", opcode=row, uops=u, rd1_en=_has_src1(spec)
            ).sha(ver)
        except Exception:
            pass
    op = DO.DveOp("EXP64_ANT", spec, subdim=False, uops_sha=shas)
    DO.OPS.append(op)
    DO._SUB_OPCODE_FOR_NAME["EXP64_ANT"] = row
    DO.CUSTOM_DVE_SPECS["EXP64_ANT"] = spec
    return op


def _build():
    EXP64 = _register_exp64()
    nc = bacc.Bacc("TRN2", target_bir_lowering=False, debug=False, num_devices=8)

    WQDT = FP8E4 if FP8_CONV else BF16
    x_d = nc.declare_dram_parameter("x", [C, HW], F32, isOutput=False)
    wq_d = nc.declare_dram_parameter("wqkvT", [C, 3 * C], WQDT, isOutput=False)
    wp_d = nc.declare_dram_parameter("wprojT", [C, C], BF16, isOutput=False)
    qkb_d = nc.declare_dram_parameter("qkb", [2 * C], F32, isOutput=False)
    vb_d = nc.declare_dram_parameter("vb", [C], BF16, isOutput=False)
    pb_d = nc.declare_dram_parameter("pb", [C], F32, isOutput=False)
    gam_d = nc.declare_dram_parameter("gamma", [C], F32, isOutput=False)
    bet_d = nc.declare_dram_parameter("beta", [C], F32, isOutput=False)
    G_d = nc.declare_dram_parameter("G", [128, GPT], F32, isOutput=False)
    GT_d = nc.declare_dram_parameter("GT", [8, 128], F32, isOutput=False)
    out_d = nc.declare_dram_parameter("out", [C, HW], F32, isOutput=True)
    h2_d = nc.dram_tensor("h2d", [C, HW], BF16)

    import bass_rust
    from contextlib import ExitStack

    with tile.TileContext(nc) as tc, ExitStack() as ctx:
        const = ctx.enter_context(tc.tile_pool(name="const", bufs=1))
        small = ctx.enter_context(tc.tile_pool(name="small", bufs=2))
        xp = ctx.enter_context(tc.tile_pool(name="xp", bufs=1))
        hp = ctx.enter_context(tc.tile_pool(name="hp", bufs=1))
        wqp = ctx.enter_context(tc.tile_pool(name="wqp", bufs=1))
        wpp = ctx.enter_context(tc.tile_pool(name="wpp", bufs=1))
        qkp = ctx.enter_context(tc.tile_pool(name="qkp", bufs=1))
        vpl = ctx.enter_context(tc.tile_pool(name="vpl", bufs=1))
        ptp = ctx.enter_context(tc.tile_pool(name="ptp", bufs=4))
        o2tp = ctx.enter_context(tc.tile_pool(name="o2tp", bufs=2))
        o2trp = ctx.enter_context(tc.tile_pool(name="o2trp", bufs=2))
        o2p = ctx.enter_context(tc.tile_pool(name="o2p", bufs=2))
        h2p = ctx.enter_context(tc.tile_pool(name="h2p", bufs=1))
        outp = ctx.enter_context(tc.tile_pool(name="outp", bufs=2))
        # PSUM: psA = scores (+ proj at drain), psB = convs/attnv; 8 banks
        psA = ctx.enter_context(tc.tile_pool(name="psA", bufs=2, space="PSUM"))
        psB = ctx.enter_context(tc.tile_pool(name="psB", bufs=2, space="PSUM"))

        # ---- ACT table preloads first: sqrt now, exp forced after last sqrt
        dummy = small.tile([1, 1], F32, tag="dummy")
        nc.vector.memset(dummy[:], 1.0)
        dummy2 = small.tile([1, 1], F32, tag="dummy2")
        nc.scalar.activation(dummy2[:], dummy[:],
                             mybir.ActivationFunctionType.Sqrt, bias=0.0, scale=1.0)

        # ---- input DMAs: x half-tiles first, then consts, then weights ----
        x_sb = xp.tile([128, CT, HW], F32)
        x_r = x_d[:].rearrange("(t p) s -> t p s", p=128)
        for t in range(CT):
            nc.sync.dma_start(out=x_sb[:, t, 0:512], in_=x_r[t][:, 0:512])
            nc.scalar.dma_start(out=x_sb[:, t, 512:1024],
                                in_=x_r[t][:, 512:1024])

        gam_sb = const.tile([128, CT], F32, tag="gam")
        nc.scalar.dma_start(out=gam_sb[:], in_=gam_d[:].rearrange("(t p) -> p t", p=128))
        bet_sb = const.tile([128, CT], F32, tag="bet")
        nc.scalar.dma_start(out=bet_sb[:], in_=bet_d[:].rearrange("(t p) -> p t", p=128))
        G_sb = const.tile([128, GPT], F32, tag="G")
        nc.scalar.dma_start(out=G_sb[:], in_=G_d[:])
        GT_sb = const.tile([8, 128], F32, tag="GT")
        nc.scalar.dma_start(out=GT_sb[:], in_=GT_d[:])
        qkb_sb = const.tile([128, 2 * CT], F32, tag="qkb")
        nc.scalar.dma_start(out=qkb_sb[:], in_=qkb_d[:].rearrange("(t p) -> p t", p=128))
        vb_sb = const.tile([1, C], BF16, tag="vb")
        nc.scalar.dma_start(out=vb_sb[:], in_=vb_d[:].rearrange("c -> () c"))
        pb_sb = const.tile([128, CT], F32, tag="pb")
        nc.scalar.dma_start(out=pb_sb[:], in_=pb_d[:].rearrange("(t p) -> p t", p=128))
        ones1 = const.tile([1, 128], BF16, tag="ones1")
        nc.vector.memset(ones1[:], 1.0)

        wq_sb = wqp.tile([128, CT, 3 * C], WQDT)
        wq_r = wq_d[:].rearrange("(t p) o -> t p o", p=128)
        for k in range(CT):
            nc.gpsimd.dma_start(out=wq_sb[:, k, :], in_=wq_r[k])
        wp_sb = wpp.tile([128, CT, C], BF16)
        nc.gpsimd.dma_start(out=wp_sb[:], in_=wp_d[:].rearrange("(t p) o -> p t o", p=128))

        # ---- per-tile groupnorm (starts as each x tile arrives) ----
        eps_sb = small.tile([8, 1], F32, tag="eps")
        nc.vector.memset(eps_sb[:], float(EPS))
        HDT = FP8E4 if FP8_CONV else BF16
        h_sb = hp.tile([128, CT, HW], HDT)
        mv = small.tile([128, CT, 3], F32, tag="mv")
        last_sqrt = None
        for t in range(CT):
            st = small.tile([128, 2, 6], F32, tag="bnst")
            x3 = x_sb[:, t, :].rearrange("p (a f) -> p a f", a=2)
            nc.vector.bn_stats(st[:, 0, :], x3[:, 0, :])
            nc.vector.bn_stats(st[:, 1, :], x3[:, 1, :])
            nc.vector.bn_aggr(mv[:, t, 0:2], st[:])
            nc.vector.tensor_mul(mv[:, t, 2:3], mv[:, t, 0:1], mv[:, t, 0:1])
            psg = psB.tile([8, 3], F32, tag="att", name=f"g_{t}")
            nc.tensor.matmul(psg[:], lhsT=G_sb[:], rhs=mv[:, t, :],
                             start=True, stop=True)
            gst = small.tile([8, 3], F32, tag="gst")
            nc.vector.tensor_copy(gst[:], psg[:])
            sqv = small.tile([8, 2], F32, tag="sqv")
            nc.vector.tensor_mul(sqv[:, 0:1], gst[:, 0:1], gst[:, 0:1])
            nc.vector.tensor_add(sqv[:, 1:2], gst[:, 1:2], gst[:, 2:3])
            nc.vector.tensor_sub(sqv[:, 1:2], sqv[:, 1:2], sqv[:, 0:1])
            srt = small.tile([8, 1], F32, tag="srt")
            last_sqrt = nc.scalar.activation(
                srt[:], sqv[:, 1:2], mybir.ActivationFunctionType.Sqrt,
                bias=eps_sb[:], scale=1.0)
            rstd = small.tile([8, 1], F32, tag="rstd")
            nc.vector.reciprocal(rstd[:], srt[:])
            gv2 = small.tile([8, 2], F32, tag="gv2")
            nc.vector.tensor_copy(gv2[:, 0:1], rstd[:])
            nc.vector.tensor_copy(gv2[:, 1:2], gst[:, 0:1])
            bc_ps = psB.tile([128, 2], F32, tag="att", name=f"bc_{t}")
            nc.tensor.matmul(bc_ps[:], lhsT=GT_sb[:], rhs=gv2[:],
                             start=True, stop=True)
            sc = small.tile([128, CT, 2], F32, tag="sc")
            nc.vector.tensor_mul(sc[:, t, 0:1], bc_ps[:, 0:1], gam_sb[:, t:t + 1])
            nc.vector.tensor_mul(sc[:, t, 1:2], bc_ps[:, 1:2], sc[:, t, 0:1])
            nc.vector.tensor_sub(sc[:, t, 1:2], bet_sb[:, t:t + 1], sc[:, t, 1:2])
            # apply per half, Vector + GpSimd in parallel
            for n, eng in ((0, nc.vector), (1, nc.gpsimd)):
                eng.tensor_scalar(
                    out=h_sb[:, t, n * 512:(n + 1) * 512],
                    in0=x_sb[:, t, n * 512:(n + 1) * 512],
                    scalar1=sc[:, t, 0:1], scalar2=sc[:, t, 1:2],
                    op0=mybir.AluOpType.mult, op1=mybir.AluOpType.add)
        # preload ACT exp table after the last sqrt (forced order so Tile
        # can't hoist it between the sqrts and thrash the table RAM)
        dummy3 = small.tile([1, 1], F32, tag="dummy3")
        expd = nc.scalar.activation(dummy3[:], dummy[:],
                                    mybir.ActivationFunctionType.Exp, scale=1.0)
        bass_rust.add_dep_helper(expd.ins, last_sqrt.ins, reason="ACT table order")

        qk_sb = qkp.tile([128, 2 * CT, HW], BF16)
        v_sb = vpl.tile([128, ST, NH * 66], FP8E4)
        nc.vector.memset(
            v_sb[:].rearrange("p m (h e) -> p m h e", e=66)[:, :, :, 64], 1.0)
        h2_sb = h2p.tile([128, CT, HW], BF16)

        if FP8_CONV:
            def emit_qk_conv(m):
                ps = psB.tile([128, HW], F32, tag="att", name=f"qkps{m}")
                for kk in (0, 2):
                    for n in range(2):
                        nc.tensor.matmul(
                            ps[:, n * 512:(n + 1) * 512],
                            lhsT=wq_sb[:, kk:kk + 2, m * 128:(m + 1) * 128],
                            rhs=h_sb[:, kk:kk + 2, n * 512:(n + 1) * 512],
                            start=(kk == 0), stop=(kk == 2),
                            perf_mode=mybir.MatmulPerfMode.DoubleRow)
                nc.scalar.add(qk_sb[:, m, :], ps[:], add=qkb_sb[:, m:m + 1])

            def emit_v_conv(m):
                psv = psB.tile([128, 512], F32, tag="att", name=f"vps{m}")
                for kk in (0, 2):
                    nc.tensor.matmul(
                        psv[:],
                        lhsT=h_sb[:, kk:kk + 2, m * 128:(m + 1) * 128],
                        rhs=wq_sb[:, kk:kk + 2, 2 * C:3 * C],
                        start=(kk == 0), stop=False,
                        perf_mode=mybir.MatmulPerfMode.DoubleRow)
                # rank-1 v-bias add: psv += ones[s] * vb[vc]
                nc.tensor.matmul(psv[:], lhsT=ones1[:], rhs=vb_sb[:],
                                 start=False, stop=True)
                nc.vector.tensor_copy(
                    v_sb[:, m, :].rearrange("p (h e) -> p h e", e=66)[:, :, 0:64],
                    psv[:].rearrange("p (h d) -> p h d", d=64))
        else:
            def emit_qk_conv(m):
                ps = psB.tile([128, HW], F32, tag="att", name=f"qkps{m}")
                for k in range(CT):
                    for n in range(2):
                        nc.tensor.matmul(
                            ps[:, n * 512:(n + 1) * 512],
                            lhsT=wq_sb[:, k, m * 128:(m + 1) * 128],
                            rhs=h_sb[:, k, n * 512:(n + 1) * 512],
                            start=(k == 0), stop=(k == CT - 1))
                nc.scalar.add(qk_sb[:, m, :], ps[:], add=qkb_sb[:, m:m + 1])

            def emit_v_conv(m):
                psv = psB.tile([128, 512], F32, tag="att", name=f"vps{m}")
                for k in range(CT):
                    nc.tensor.matmul(
                        psv[:],
                        lhsT=h_sb[:, k, m * 128:(m + 1) * 128],
                        rhs=wq_sb[:, k, 2 * C:3 * C],
                        start=(k == 0), stop=False)
                nc.tensor.matmul(psv[:], lhsT=ones1[:], rhs=vb_sb[:],
                                 start=False, stop=True)
                nc.vector.tensor_copy(
                    v_sb[:, m, :].rearrange("p (h e) -> p h e", e=66)[:, :, 0:64],
                    psv[:].rearrange("p (h d) -> p h d", d=64))

        # q/k tiles for pair 0 first, so its scores can start immediately
        emit_qk_conv(0)
        emit_qk_conv(4)
        # remaining conv work, interleaved into pair 0's attnv slot below
        conv_work = [lambda m=m: emit_qk_conv(m) for m in (1, 5, 2, 6, 3, 7)]
        conv_work += [lambda m=m: emit_v_conv(m) for m in range(ST)]

        def emit_scores_step(cur, step):
            pss = []
            for (h, pt) in cur:
                base = 64 * (h % 2)
                ps = psA.tile([128, HW], F32, tag="sc", name=f"scps{h}_{step}")
                pss.append(ps)
                kT = qk_sb[base:base + 64, CT + h // 2,
                           step * 128:(step + 1) * 128]
                qT = qk_sb[base:base + 64, h // 2, :]
                for n in range(2):
                    nc.tensor.matmul(
                        ps[:, n * 512:(n + 1) * 512], lhsT=kT,
                        rhs=qT[:, n * 512:(n + 1) * 512],
                        start=True, stop=True)
            (hA, ptA), (hB, ptB) = cur
            nc.scalar.activation(
                ptA[:, step, :], pss[0][:],
                mybir.ActivationFunctionType.Exp,
                scale=float(DH ** -0.5))
            nc.vector._custom_dve(
                EXP64, out=ptB[:, step, :], in0=pss[1][:],
                s0=float(DH ** -0.5) / 64.0)

        def emit_attnv_sm(h, pt, sm, state):
            # DoubleRow packs a j-tile pair per matmul (fp8 weights
            # 2-per-cell, K=256 virtual)
            if sm == 0:
                state[h] = psB.tile([128, HW], F32, tag="att", name=f"po{h}")
            po = state[h]
            jj = 2 * sm
            v2_ = v_sb[:].rearrange(
                "p m (hh e) -> p m hh e", e=66)[:, jj:jj + 2, h, 0:65]
            for n in range(2):
                nc.tensor.matmul(
                    po[0:65, n * 512:(n + 1) * 512],
                    lhsT=v2_,
                    rhs=pt[:, jj:jj + 2, n * 512:(n + 1) * 512],
                    start=(sm == 0), stop=(sm == 3),
                    perf_mode=mybir.MatmulPerfMode.DoubleRow)
            if sm == 3:
                o2t = o2tp.tile([80, HW], BF16, tag="o2t")
                nc.scalar.copy(o2t[0:65, :], po[0:65, :])
                o2tr = o2trp.tile([128, ST, 80], BF16, tag="o2tr")
                nc.sync.dma_start_transpose(o2tr[:], o2t[:])
                linv = small.tile([128, ST], F32, tag="linv")
                nc.vector.reciprocal(linv[:], o2tr[:, :, 64])
                o2 = o2p.tile([128, 512], BF16, tag="o2")
                lap = linv[:]
                lbc = bass.AP(tensor=lap.tensor, offset=lap.offset,
                              ap=[[lap.ap[0][0], 128], [1, ST], [0, 64]])
                nc.gpsimd.tensor_mul(
                    o2[:].rearrange("p (q d) -> p q d", d=64),
                    o2tr[:, :, 0:64], lbc)
                wr = nc.sync.dma_start(
                    out=h2_d[:].rearrange("c s -> (c s)")
                    [h * 65536:(h + 1) * 65536]
                    .rearrange("(q p d) -> p q d", p=128, d=64),
                    in_=o2[:].rearrange("p (q d) -> p q d", d=64))
                state.setdefault("wr", []).append(wr)
                # read back this head's 64 h2 rows right away
                k, half = h // 2, h % 2
                rd = nc.sync.dma_start(
                    out=h2_sb[64 * half:64 * half + 64, k, :],
                    in_=h2_d[h * 64:(h + 1) * 64, :])
                bass_rust.add_dep_helper(rd.ins, wr.ins, reason="h2 RAW")

        proj_pp = {}

        def emit_proj(o, ks, finish, pool=None, tag="sc"):
            if o not in proj_pp:
                proj_pp[o] = (pool or psA).tile([128, HW], F32, tag=tag,
                                                name=f"pp{o}")
            pp = proj_pp[o]
            for k in ks:
                for n in range(2):
                    nc.tensor.matmul(
                        pp[:, n * 512:(n + 1) * 512],
                        lhsT=wp_sb[:, k, o * 128:(o + 1) * 128],
                        rhs=h2_sb[:, k, n * 512:(n + 1) * 512],
                        start=(k == 0), stop=(k == CT - 1))
            if finish:
                # out = (proj + proj_b) + x  in one fused DVE op
                ot = outp.tile([128, HW], F32, tag="ot")
                nc.vector.scalar_tensor_tensor(
                    out=ot[:], in0=pp[:], scalar=pb_sb[:, o:o + 1],
                    in1=x_sb[:, o, :],
                    op0=mybir.AluOpType.add, op1=mybir.AluOpType.add)
                eng = nc.sync if o % 2 == 0 else nc.scalar
                eng.dma_start(out=out_d[o * 128:(o + 1) * 128, :], in_=ot[:])
                del proj_pp[o]

        # ---- attention pair loop (software pipelined, pairs 0-3) ----
        prev = None
        for hp_i in range(4):
            hA, hB = 2 * hp_i, 2 * hp_i + 1
            ptA = ptp.tile([128, ST, HW], FP8E5, tag="pt", name=f"pt{hA}")
            ptB = ptp.tile([128, ST, HW], FP8E5, tag="pt", name=f"pt{hB}")
            cur = [(hA, ptA), (hB, ptB)]
            state = {}
            for step in range(8):
                emit_scores_step(cur, step)
                if prev is not None:
                    h, pt = prev[step // 4]
                    emit_attnv_sm(h, pt, step % 4, state)
                elif conv_work:
                    # pair 0: fill the attnv slot with remaining conv tiles
                    conv_work.pop(0)()
                    if conv_work and step % 2 == 1:
                        conv_work.pop(0)()
            while prev is None and conv_work:
                conv_work.pop(0)()
            prev = cur

        # ---- drain: last pair's attnv at double rate, proj spread under
        # the h2 round-trip latency so the PE stays warm ----
        (h6, pt6), (h7, pt7) = prev
        state = {}
        for sm in range(4):
            emit_attnv_sm(h6, pt6, sm, state)
            emit_attnv_sm(h7, pt7, sm, state)
            if sm == 1:
                emit_proj(0, [0], finish=False)
            if sm == 2:
                emit_proj(1, [0], finish=False)
        emit_proj(0, [1], finish=False)
        emit_proj(1, [1], finish=False)
        emit_proj(0, [2], finish=False)
        emit_proj(1, [2], finish=False)
        emit_proj(2, [0], finish=False, pool=psB, tag="att")
        emit_proj(2, [1], finish=False, pool=psB, tag="att")
        emit_proj(3, [0], finish=False, pool=psB, tag="att")
        emit_proj(2, [2], finish=False, pool=psB, tag="att")
        emit_proj(3, [1], finish=False, pool=psB, tag="att")
        emit_proj(3, [2], finish=False, pool=psB, tag="att")

        # ---- proj finish ----
        emit_proj(0, [3], finish=True)
        emit_proj(1, [3], finish=True)
        emit_proj(2, [3], finish=True)
        emit_proj(3, [3], finish=True)

    nc.compile()
    return nc


def _host_prep(x, norm_gamma, norm_beta, qkv_w, qkv_b, proj_w, proj_b):
    x = np.asarray(x, dtype=np.float32).reshape(B, C, HW)
    qkv_w = np.asarray(qkv_w, dtype=np.float32)
    qkv_b = np.asarray(qkv_b, dtype=np.float32)
    proj_w = np.asarray(proj_w, dtype=np.float32)
    proj_b = np.asarray(proj_b, dtype=np.float32)

    wq_np = np.ascontiguousarray(qkv_w.T)
    if FP8_CONV:
        wqkvT = np.clip(wq_np, -440.0, 440.0).astype(ml_dtypes.float8_e4m3fn)
    else:
        wqkvT = wq_np.astype(ml_dtypes.bfloat16)
    wprojT = np.ascontiguousarray(proj_w.T).astype(ml_dtypes.bfloat16)
    qkb = np.ascontiguousarray(qkv_b[:2 * C])
    vb = np.ascontiguousarray(qkv_b[2 * C:]).astype(ml_dtypes.bfloat16)
    pb = np.ascontiguousarray(proj_b, dtype=np.float32)

    G = np.zeros((128, GPT), np.float32)
    for p in range(128):
        G[p, p // CPG] = 1.0 / CPG
    GT = np.zeros((8, 128), np.float32)
    for p in range(128):
        GT[p // CPG, p] = 1.0

    gamma = np.ascontiguousarray(norm_gamma, dtype=np.float32)
    beta = np.ascontiguousarray(norm_beta, dtype=np.float32)

    in_maps = []
    for b in range(B):
        in_maps.append({
            "x": np.ascontiguousarray(x[b]),
            "wqkvT": wqkvT, "wprojT": wprojT,
            "qkb": qkb, "vb": vb, "pb": pb,
            "gamma": gamma, "beta": beta,
            "G": G, "GT": GT,
        })
    return in_maps


def _run(inputs: dict, trace: bool = False, tmpdir=None):
    if "nc" not in _CACHE:
        _CACHE["nc"] = _build()
    nc = _CACHE["nc"]
    in_maps = _host_prep(**inputs)
    res = run_bass_kernel_spmd(nc, in_maps, core_ids=list(range(8)), trace=trace,
                               tmpdir=tmpdir)
    out = np.stack([r["out"] for r in res.results]).reshape(B, C, 32, 32)
    return out.astype(np.float32), res


def kernel(**inputs):
    out, _ = _run(inputs, trace=False)
    return out
